# revision 10
# baseline (speedup 1.0000x reference)
"""Trilinear grid_pull on 8 Trainium2 cores.

Core c handles batch b=c//4 and output-grid x-slab xq=c%4 (32 planes).
Phase 1 packs input[b] into a bf16 table G[v, 8] with v=(x*128+y)*128+z and
entries (dx*4+dy*2+c) = vol[c, x+dx, y+dy, z]; one indirect-DMA descriptor at
row v then covers rows v and v+1 = all 16 trilinear taps (2ch x 8 corners).
Phase 2 streams 128 single-column indirect gathers per output plane (the only
reliable indirect shape on this HW), then lerps whole planes on the vector
engine, overlapped with the gather stream.
"""
import numpy as np
from contextlib import ExitStack

from concourse import bass, bacc, mybir
import concourse.tile as tile
from concourse.bass_utils import run_bass_kernel_spmd

P = 128
N = 128           # volume side
C = 2             # channels
B = 2             # batch
XS = 32           # x-planes per core (output slab)
V = N * N * N     # packed blocks
F32 = mybir.dt.float32
BF16 = mybir.dt.bfloat16
I32 = mybir.dt.int32

_CACHE = {}
TRACE = False


def build_kernel():
    nc = bacc.Bacc('TRN2', target_bir_lowering=False, num_devices=8)
    inp = nc.dram_tensor("inp", [C, N, N, N], F32, kind="ExternalInput")
    g3 = nc.dram_tensor("g3", [3, XS, N, N], F32, kind="ExternalInput")
    out = nc.dram_tensor("o", [C, XS, N, N], F32, kind="ExternalOutput")
    G = nc.dram_tensor("G", [V, 8], BF16)  # packed taps

    inp_f = inp[:]
    G_f = G[:]

    XG = 16  # x-columns per build group

    with tile.TileContext(nc) as tc:
        with ExitStack() as ctx:
            # ---------------- Phase 1: build G (bf16) ----------------
            # partition = x dataflow: contiguous loads, SBUF->SBUF x+1 shift,
            # dy as free-dim offset, contiguous 32KB-run G writes.
            lp = ctx.enter_context(tc.tile_pool(name="load", bufs=3))
            gp = ctx.enter_context(tc.tile_pool(name="gx", bufs=3))
            YC = 8   # y rows per chunk

            for yc in range(N // YC):
                y0 = yc * YC
                nyl = YC + 1 if yc < N // YC - 1 else YC
                t = {}
                for c in range(C):
                    tv = lp.tile([P, YC + 1, N], F32, tag=f"v{c}")
                    src = inp_f[c, :, y0:y0 + nyl, :]
                    eng = nc.sync if c == 0 else nc.scalar
                    eng.dma_start(tv[:, 0:nyl, :], src)
                    t[(c, 0)] = tv
                    # x+1 shifted copy via SBUF->SBUF DMA
                    ts = lp.tile([P, YC + 1, N], F32, tag=f"s{c}")
                    eng2 = nc.scalar if c == 0 else nc.sync
                    eng2.dma_start(ts[0:P - 1, 0:nyl, :], tv[1:P, 0:nyl, :])
                    t[(c, 1)] = ts
                gx = gp.tile([P, YC, N, 8], BF16, tag="gx")
                for dx in range(2):
                    for dy in range(2):
                        for c in range(C):
                            k = dx * 4 + dy * 2 + c
                            nyw = YC if (yc < N // YC - 1 or dy == 0) else YC - 1
                            src = t[(c, dx)][:, dy:dy + nyw, :]
                            nc.vector.tensor_copy(gx[:, 0:nyw, :, k], src)
                dst = G_f.rearrange("(x y z) k -> x y z k", x=N, y=N)[
                    :, y0:y0 + YC, :, :]
                eng = nc.sync if yc % 2 == 0 else nc.scalar
                eng.dma_start(dst, gx[:])

            # ---------------- Phase 2: gather + lerp ----------------
            pp = ctx.enter_context(tc.tile_pool(name="plane", bufs=2))
            wp = ctx.enter_context(tc.tile_pool(name="wts", bufs=2))
            bp = ctx.enter_context(tc.tile_pool(name="gath", bufs=3))
            vp = ctx.enter_context(tc.tile_pool(name="lerp", bufs=1))
            op = ctx.enter_context(tc.tile_pool(name="outp", bufs=2))

            XOG = 8  # planes per grid-load / out-store group
            for xog in range(XS // XOG):
                gt = {}
                for d in range(3):
                    t = pp.tile([P, XOG, N], F32, tag=f"g{d}")
                    src = g3[d, xog * XOG:(xog + 1) * XOG, :, :].transpose([1, 0, 2])
                    nc.sync.dma_start(t[:], src)
                    gt[d] = t
                oc = {}
                for c in range(C):
                    oc_t = op.tile([P, XOG, N], F32, tag=f"oc{c}")
                    oc[c] = oc_t

                for xo in range(XOG):
                    cc = {d: gt[d][:, xo, :] for d in range(3)}
                    ff = {}
                    w = {}
                    for d in range(3):
                        # floor via round-to-nearest(g - 0.5); clamp to [0,126]
                        ti = wp.tile([P, N], I32, tag=f"ti{d}")
                        nc.vector.tensor_scalar(ti[:], cc[d], 0.5, None,
                                                mybir.AluOpType.subtract)
                        tfc = wp.tile([P, N], F32, tag=f"tfc{d}")
                        nc.vector.tensor_scalar(tfc[:], ti[:], 0, 126,
                                                mybir.AluOpType.max,
                                                mybir.AluOpType.min)
                        wd = wp.tile([P, N], F32, tag=f"w{d}")
                        nc.vector.tensor_sub(wd[:], cc[d], tfc[:])
                        ff[d] = tfc
                        w[d] = wd
                    # idx = (fx*128 + fy)*128 + fz  (fp32 exact), then cast
                    t1 = wp.tile([P, N], F32, tag="t1")
                    nc.vector.scalar_tensor_tensor(
                        t1[:], ff[0][:], 128.0, ff[1][:],
                        mybir.AluOpType.mult, mybir.AluOpType.add)
                    t2 = wp.tile([P, N], F32, tag="t2")
                    nc.vector.scalar_tensor_tensor(
                        t2[:], t1[:], 128.0, ff[2][:],
                        mybir.AluOpType.mult, mybir.AluOpType.add)
                    idx = wp.tile([P, N], I32, tag="idx")
                    nc.vector.tensor_copy(idx[:], t2[:])

                    # gather: one [128,1] instr per z-column (whole plane)
                    gb = bp.tile([P, N, 16], BF16, tag="gb")
                    for z in range(N):
                        nc.gpsimd.indirect_dma_start(
                            out=gb[:, z, :],
                            out_offset=None,
                            in_=G_f,
                            in_offset=bass.IndirectOffsetOnAxis(
                                ap=idx[:, z:z + 1], axis=0),
                        )

                    def bc(ap, reps):
                        return ap.unsqueeze(2).broadcast_to([P, N, reps])

                    vz = vp.tile([P, N, 8], F32, tag="vz")
                    nc.vector.tensor_sub(vz[:], gb[:, :, 8:16], gb[:, :, 0:8])
                    nc.vector.tensor_mul(vz[:], vz[:], bc(w[2], 8))
                    nc.vector.tensor_add(vz[:], vz[:], gb[:, :, 0:8])

                    vx = vp.tile([P, N, 4], F32, tag="vx")
                    nc.vector.tensor_sub(vx[:], vz[:, :, 4:8], vz[:, :, 0:4])
                    nc.vector.tensor_mul(vx[:], vx[:], bc(w[0], 4))
                    nc.vector.tensor_add(vx[:], vx[:], vz[:, :, 0:4])

                    vy = vp.tile([P, N, 2], F32, tag="vy")
                    nc.vector.tensor_sub(vy[:], vx[:, :, 2:4], vx[:, :, 0:2])
                    nc.vector.tensor_mul(vy[:], vy[:], bc(w[1], 2))
                    nc.vector.tensor_add(vy[:], vy[:], vx[:, :, 0:2])

                    for c in range(C):
                        nc.vector.tensor_copy(oc[c][:, xo, :], vy[:, :, c])

                for c in range(C):
                    dst = out[:][c, xog * XOG:(xog + 1) * XOG, :, :].transpose([1, 0, 2])
                    nc.sync.dma_start(dst, oc[c][:])

    nc.compile()
    return nc


def kernel(input, grid):
    input = np.ascontiguousarray(input, dtype=np.float32)
    grid = np.ascontiguousarray(grid, dtype=np.float32)
    key = "nc"
    if key not in _CACHE:
        _CACHE[key] = build_kernel()
    nc = _CACHE[key]
    in_maps = []
    for core in range(8):
        b, xq = core // 4, core % 4
        in_maps.append({
            "inp": input[b],
            "g3": np.ascontiguousarray(grid[b, :, xq * XS:(xq + 1) * XS]),
        })
    res = run_bass_kernel_spmd(nc, in_maps, core_ids=list(range(8)), trace=TRACE)
    if TRACE and res.exec_time_ns is not None:
        print(f"HW exec time: {res.exec_time_ns} ns")
        globals()["LAST_EXEC_NS"] = res.exec_time_ns
        globals()["LAST_RESULTS"] = res
    out = np.empty((B, C, N, N, N), dtype=np.float32)
    for core in range(8):
        b, xq = core // 4, core % 4
        out[b, :, xq * XS:(xq + 1) * XS] = res.results[core]["o"]
    return out


if __name__ == "__main__":
    rng = np.random.default_rng(0)
    inp = rng.standard_normal((B, C, N, N, N)).astype(np.float32)
    grid = (rng.random((B, 3, N, N, N), dtype=np.float32) * (N - 1)).astype(np.float32)
    got = kernel(inp, grid)
    print(got.shape, got.dtype)


# revision 12
# speedup vs baseline: 1.0103x; 1.0103x over previous
"""Trilinear grid_pull on 8 Trainium2 cores.

Core c handles batch b=c//4 and output-grid x-slab xq=c%4 (32 planes).
Phase 1 packs input[b] into a bf16 table G[v, 8] with v=(x*128+y)*128+z and
entries (dx*4+dy*2+c) = vol[c, x+dx, y+dy, z]; one indirect-DMA descriptor at
row v then covers rows v and v+1 = all 16 trilinear taps (2ch x 8 corners).
Phase 2 streams 128 single-column indirect gathers per output plane (the only
reliable indirect shape on this HW), then lerps whole planes on the vector
engine, overlapped with the gather stream.
"""
import numpy as np
from contextlib import ExitStack

from concourse import bass, bacc, mybir
import concourse.tile as tile
from concourse.bass_utils import run_bass_kernel_spmd

P = 128
N = 128           # volume side
C = 2             # channels
B = 2             # batch
XS = 32           # x-planes per core (output slab)
V = N * N * N     # packed blocks
F32 = mybir.dt.float32
BF16 = mybir.dt.bfloat16
I32 = mybir.dt.int32

_CACHE = {}
TRACE = False


def build_kernel():
    nc = bacc.Bacc('TRN2', target_bir_lowering=False, num_devices=8)
    inp = nc.dram_tensor("inp", [C, N, N, N], F32, kind="ExternalInput")
    g3 = nc.dram_tensor("g3", [3, XS, N, N], F32, kind="ExternalInput")
    out = nc.dram_tensor("o", [C, XS, N, N], F32, kind="ExternalOutput")
    G = nc.dram_tensor("G", [V, 8], BF16)  # packed taps

    inp_f = inp[:]
    G_f = G[:]

    XG = 16  # x-columns per build group

    with tile.TileContext(nc) as tc:
        with ExitStack() as ctx:
            # ---------------- Phase 1: build G (bf16) ----------------
            # partition = x dataflow: contiguous loads, SBUF->SBUF x+1 shift,
            # dy as free-dim offset, contiguous 32KB-run G writes.
            lp = ctx.enter_context(tc.tile_pool(name="load", bufs=2))
            gp = ctx.enter_context(tc.tile_pool(name="gx", bufs=2))
            YC = 16  # y rows per chunk

            for yc in range(N // YC):
                y0 = yc * YC
                nyl = YC + 1 if yc < N // YC - 1 else YC
                t = {}
                for c in range(C):
                    tv = lp.tile([P, YC + 1, N], F32, tag=f"v{c}")
                    src = inp_f[c, :, y0:y0 + nyl, :]
                    eng = nc.sync if c == 0 else nc.scalar
                    eng.dma_start(tv[:, 0:nyl, :], src)
                    t[(c, 0)] = tv
                    # x+1 shifted copy loaded directly from DRAM in parallel
                    ts = lp.tile([P, YC + 1, N], F32, tag=f"s{c}")
                    eng2 = nc.scalar if c == 0 else nc.sync
                    eng2.dma_start(ts[0:P - 1, 0:nyl, :],
                                   inp_f[c, 1:P, y0:y0 + nyl, :])
                    t[(c, 1)] = ts
                gx = gp.tile([P, YC, N, 8], BF16, tag="gx")
                for dx in range(2):
                    for dy in range(2):
                        for c in range(C):
                            k = dx * 4 + dy * 2 + c
                            nyw = YC if (yc < N // YC - 1 or dy == 0) else YC - 1
                            src = t[(c, dx)][:, dy:dy + nyw, :]
                            nc.vector.tensor_copy(gx[:, 0:nyw, :, k], src)
                dst = G_f.rearrange("(x y z) k -> x y z k", x=N, y=N)[
                    :, y0:y0 + YC, :, :]
                eng = nc.sync if yc % 2 == 0 else nc.scalar
                eng.dma_start(dst, gx[:])

            # ---------------- Phase 2: gather + lerp ----------------
            pp = ctx.enter_context(tc.tile_pool(name="plane", bufs=2))
            wp = ctx.enter_context(tc.tile_pool(name="wts", bufs=2))
            bp = ctx.enter_context(tc.tile_pool(name="gath", bufs=3))
            vp = ctx.enter_context(tc.tile_pool(name="lerp", bufs=1))
            op = ctx.enter_context(tc.tile_pool(name="outp", bufs=2))

            XOG = 8  # planes per grid-load / out-store group
            for xog in range(XS // XOG):
                gt = {}
                for d in range(3):
                    t = pp.tile([P, XOG, N], F32, tag=f"g{d}")
                    src = g3[d, xog * XOG:(xog + 1) * XOG, :, :].transpose([1, 0, 2])
                    nc.sync.dma_start(t[:], src)
                    gt[d] = t
                oc = {}
                for c in range(C):
                    oc_t = op.tile([P, XOG, N], F32, tag=f"oc{c}")
                    oc[c] = oc_t

                for xo in range(XOG):
                    cc = {d: gt[d][:, xo, :] for d in range(3)}
                    ff = {}
                    w = {}
                    for d in range(3):
                        # floor via round-to-nearest(g - 0.5); clamp to [0,126]
                        ti = wp.tile([P, N], I32, tag=f"ti{d}")
                        nc.vector.tensor_scalar(ti[:], cc[d], 0.5, None,
                                                mybir.AluOpType.subtract)
                        tfc = wp.tile([P, N], F32, tag=f"tfc{d}")
                        nc.vector.tensor_scalar(tfc[:], ti[:], 0, 126,
                                                mybir.AluOpType.max,
                                                mybir.AluOpType.min)
                        wd = wp.tile([P, N], F32, tag=f"w{d}")
                        nc.vector.tensor_sub(wd[:], cc[d], tfc[:])
                        ff[d] = tfc
                        w[d] = wd
                    # idx = (fx*128 + fy)*128 + fz  (fp32 exact), then cast
                    t1 = wp.tile([P, N], F32, tag="t1")
                    nc.vector.scalar_tensor_tensor(
                        t1[:], ff[0][:], 128.0, ff[1][:],
                        mybir.AluOpType.mult, mybir.AluOpType.add)
                    t2 = wp.tile([P, N], F32, tag="t2")
                    nc.vector.scalar_tensor_tensor(
                        t2[:], t1[:], 128.0, ff[2][:],
                        mybir.AluOpType.mult, mybir.AluOpType.add)
                    idx = wp.tile([P, N], I32, tag="idx")
                    nc.vector.tensor_copy(idx[:], t2[:])

                    # gather: one [128,1] instr per z-column (whole plane)
                    gb = bp.tile([P, N, 16], BF16, tag="gb")
                    for z in range(N):
                        nc.gpsimd.indirect_dma_start(
                            out=gb[:, z, :],
                            out_offset=None,
                            in_=G_f,
                            in_offset=bass.IndirectOffsetOnAxis(
                                ap=idx[:, z:z + 1], axis=0),
                        )

                    def bc(ap, reps):
                        return ap.unsqueeze(2).broadcast_to([P, N, reps])

                    vz = vp.tile([P, N, 8], F32, tag="vz")
                    nc.vector.tensor_sub(vz[:], gb[:, :, 8:16], gb[:, :, 0:8])
                    nc.vector.tensor_mul(vz[:], vz[:], bc(w[2], 8))
                    nc.vector.tensor_add(vz[:], vz[:], gb[:, :, 0:8])

                    vx = vp.tile([P, N, 4], F32, tag="vx")
                    nc.vector.tensor_sub(vx[:], vz[:, :, 4:8], vz[:, :, 0:4])
                    nc.vector.tensor_mul(vx[:], vx[:], bc(w[0], 4))
                    nc.vector.tensor_add(vx[:], vx[:], vz[:, :, 0:4])

                    vy = vp.tile([P, N, 2], F32, tag="vy")
                    nc.vector.tensor_sub(vy[:], vx[:, :, 2:4], vx[:, :, 0:2])
                    nc.vector.tensor_mul(vy[:], vy[:], bc(w[1], 2))
                    nc.vector.tensor_add(vy[:], vy[:], vx[:, :, 0:2])

                    for c in range(C):
                        nc.vector.tensor_copy(oc[c][:, xo, :], vy[:, :, c])

                for c in range(C):
                    dst = out[:][c, xog * XOG:(xog + 1) * XOG, :, :].transpose([1, 0, 2])
                    nc.sync.dma_start(dst, oc[c][:])

    nc.compile()
    return nc


def kernel(input, grid):
    input = np.ascontiguousarray(input, dtype=np.float32)
    grid = np.ascontiguousarray(grid, dtype=np.float32)
    key = "nc"
    if key not in _CACHE:
        _CACHE[key] = build_kernel()
    nc = _CACHE[key]
    in_maps = []
    for core in range(8):
        b, xq = core // 4, core % 4
        in_maps.append({
            "inp": input[b],
            "g3": np.ascontiguousarray(grid[b, :, xq * XS:(xq + 1) * XS]),
        })
    res = run_bass_kernel_spmd(nc, in_maps, core_ids=list(range(8)), trace=TRACE)
    if TRACE and res.exec_time_ns is not None:
        print(f"HW exec time: {res.exec_time_ns} ns")
        globals()["LAST_EXEC_NS"] = res.exec_time_ns
        globals()["LAST_RESULTS"] = res
    out = np.empty((B, C, N, N, N), dtype=np.float32)
    for core in range(8):
        b, xq = core // 4, core % 4
        out[b, :, xq * XS:(xq + 1) * XS] = res.results[core]["o"]
    return out


if __name__ == "__main__":
    rng = np.random.default_rng(0)
    inp = rng.standard_normal((B, C, N, N, N)).astype(np.float32)
    grid = (rng.random((B, 3, N, N, N), dtype=np.float32) * (N - 1)).astype(np.float32)
    got = kernel(inp, grid)
    print(got.shape, got.dtype)
